# revision 3
# baseline (speedup 1.0000x reference)
"""Trainium2 Bass kernel for nn_CAModel (neural cellular automaton step).

Computation (per image, fp32):
  pre_life = maxpool3x3(x[...,3]) > 0.1        (HOST: exact fp32)
  gx, gy   = depthwise 3x3 sobel convs of x
  perc     = interleave([x, gx, gy])            # [H,W,48]
  h        = relu(perc @ w0)                    # [H,W,128]
  dx       = h @ w1                             # [H,W,16]
  x_mid    = x + dx * (update_rand <= 0.5)      (update mask from HOST)
  life     = pre_life & (maxpool3x3(x_mid[...,3]) > 0.1)
  x_new    = x_mid * life
  returns (x_new, dx)

Mapping: 8 NeuronCores, data-parallel over batch (2 images/core).

Device pipeline per 16-row tile (2 psum-halves q of 4 row-pair groups):
  fc0: sobel folded into weights; two K-stacked matmuls per row-pair
       (taps dx=0,1 stacked on 96 partitions, tap dx=2 separate) instead
       of three -> 2 streamed columns/pixel on the PE.
  relu: split across ACT and DVE engines (psum -> bf16 sbuf).
  fc1: w1-stationary (16-col weight loads), 4 row-pair groups packed
       into one PSUM tile at col-group offsets 0/32/64/96.
  dx transpose: channel-major [16,512] slabs -> pixel-major via HWDGE
       xbar DMA transpose (bf16).
  tail (gpsimd): x_mid = x + dx*um; alpha extract; per-image maxpool
       (vertical on DVE, horizontal via bf16 shift matmuls on PE),
       life mult, x_new store.  All I/O in bf16.
"""

import functools
import os
import sys

import numpy as np

_TRN_REPO = os.environ.get("TRN_RL_REPO", "/opt/trn_rl_repo")
if _TRN_REPO not in sys.path:
    sys.path.insert(0, _TRN_REPO)

import concourse.bass as bass
import concourse.bacc as bacc
import concourse.tile as tile
from concourse import mybir
from concourse.bass_utils import run_bass_kernel_spmd

F32 = mybir.dt.float32
BF16 = mybir.dt.bfloat16
BF16_NP = mybir.dt.np(mybir.dt.bfloat16)

C = 16          # channels
HID = 128       # hidden dim
PW = 128        # partitions used as w-position within a half
N_CORES = 8
FIRE_RATE = 0.5
ALIVE_THR = 0.1

LAST_RESULTS = None  # BassKernelResults of the most recent kernel() call


# ---------------------------------------------------------------------------
# device program
# ---------------------------------------------------------------------------

def build_program(NI, H, W, TR=16):
    """Build the Bass program for one core processing NI images of HxW."""
    NH = W // PW                  # halves per row (2)
    assert W % PW == 0 and H % TR == 0 and TR == 16 and NH == 2
    Hp, Wp = H + 2, W + 2
    NRH = NI * H * NH             # total (img,row,half) count
    NT = NI * H // TR             # tiles

    nc = bacc.Bacc(trn_type="TRN2")

    xch = nc.dram_tensor("xch", [NI * C * Hp + 1, Wp], BF16, kind="ExternalInput")
    xpx_d = nc.dram_tensor("xpx", [PW, NRH, C], BF16, kind="ExternalInput")
    um_d = nc.dram_tensor("um", [PW, NRH], BF16, kind="ExternalInput")
    plx_d = nc.dram_tensor("plx", [PW, NRH], BF16, kind="ExternalInput")
    b01_d = nc.dram_tensor("b01", [96, HID], BF16, kind="ExternalInput")
    b2_d = nc.dram_tensor("b2", [48, HID], BF16, kind="ExternalInput")
    w1_d = nc.dram_tensor("w1d", [HID, C], BF16, kind="ExternalInput")
    se_d = nc.dram_tensor("SEd", [PW, PW], BF16, kind="ExternalInput")
    sw_d = nc.dram_tensor("SWd", [PW, PW], BF16, kind="ExternalInput")
    dxo = nc.dram_tensor("dxo", [NT, PW, 2 * 512], BF16, kind="ExternalOutput")
    xno = nc.dram_tensor("xno", [PW, NRH, C], BF16, kind="ExternalOutput")

    with tile.TileContext(nc) as tc:
        _emit(tc, locals())
    nc.compile()
    return nc


def _emit(tc, t):
    nc = tc.nc
    NI, H, W, TR = t["NI"], t["H"], t["W"], t["TR"]
    NH, Hp, Wp, NRH, NT = t["NH"], t["Hp"], t["Wp"], t["NRH"], t["NT"]
    xch, xpx_d, um_d, plx_d = t["xch"], t["xpx_d"], t["um_d"], t["plx_d"]
    b01_d, b2_d, w1_d, se_d, sw_d = (
        t["b01_d"], t["b2_d"], t["w1_d"], t["se_d"], t["sw_d"])
    dxo, xno = t["dxo"], t["xno"]
    HNH = H * NH                  # per-image rh span (512)
    AL = mybir.AluOpType
    Relu = mybir.ActivationFunctionType.Relu

    from contextlib import ExitStack
    ctx = ExitStack()
    with ctx:
        singles = ctx.enter_context(tc.tile_pool(name="singles", bufs=1))
        xc_pool = ctx.enter_context(tc.tile_pool(name="xc", bufs=2))
        hs_pool = ctx.enter_context(tc.tile_pool(name="hs", bufs=8))
        xp_pool = ctx.enter_context(tc.tile_pool(name="xp", bufs=2))
        dxc_pool = ctx.enter_context(tc.tile_pool(name="dxc", bufs=2))
        dxt_pool = ctx.enter_context(tc.tile_pool(name="dxt", bufs=2))
        dxm_pool = ctx.enter_context(tc.tile_pool(name="dxm", bufs=2))
        xns_pool = ctx.enter_context(tc.tile_pool(name="xns", bufs=2))
        ps_h = ctx.enter_context(tc.tile_pool(name="ps_h", bufs=5, space="PSUM"))
        ps_dx = ctx.enter_context(tc.tile_pool(name="ps_dx", bufs=2, space="PSUM"))
        ps_scr = ctx.enter_context(tc.tile_pool(name="ps_scr", bufs=1, space="PSUM"))

        # ---- constants / weights / masks ----
        b01_sb = singles.tile([96, HID], BF16)
        nc.sync.dma_start(out=b01_sb, in_=b01_d.ap())
        b2_sb = singles.tile([48, HID], BF16)
        nc.sync.dma_start(out=b2_sb, in_=b2_d.ap())
        w1_sb = singles.tile([HID, C], BF16)
        nc.sync.dma_start(out=w1_sb, in_=w1_d.ap())
        se_sb = singles.tile([PW, PW], BF16)
        nc.sync.dma_start(out=se_sb, in_=se_d.ap())
        sw_sb = singles.tile([PW, PW], BF16)
        nc.sync.dma_start(out=sw_sb, in_=sw_d.ap())
        um_sb = singles.tile([PW, NRH], BF16)
        nc.sync.dma_start(out=um_sb, in_=um_d.ap())
        plx_sb = singles.tile([PW, NRH], BF16)
        nc.sync.dma_start(out=plx_sb, in_=plx_d.ap())

        # ---- residents / mask scratch ----
        xmid = singles.tile([PW, NRH, C], BF16)
        am = singles.tile([PW, NRH], BF16)
        vm = singles.tile([PW, HNH], BF16)
        vm2 = singles.tile([PW, HNH], BF16)
        m3 = singles.tile([PW, HNH], BF16)
        life = singles.tile([PW, HNH], BF16)
        seam = singles.tile([PW, 2 * H], BF16)
        nc.vector.memset(seam, 0.0)

        # ---- PE pre-sync dummies (touch each DMA'd matmul operand once) ----
        scr = ps_scr.tile([PW, 2], F32, tag="scr")
        nc.tensor.matmul(out=scr, lhsT=b01_sb, rhs=b01_sb[:, 0:2],
                         start=True, stop=True)
        nc.tensor.matmul(out=scr, lhsT=b2_sb, rhs=b2_sb[:, 0:2],
                         start=True, stop=True)
        scr2f = ps_scr.tile([PW, 2], F32, tag="scr")
        nc.tensor.matmul(out=scr2f[0:C, :], lhsT=w1_sb, rhs=w1_sb[:, 0:2],
                         start=True, stop=True)
        nc.tensor.matmul(out=scr, lhsT=se_sb, rhs=se_sb[:, 0:2],
                         start=True, stop=True)
        nc.tensor.matmul(out=scr, lhsT=sw_sb, rhs=sw_sb[:, 0:2],
                         start=True, stop=True)

        # ---- prime ps_dx garbage lanes so full-tile reads are defined ----
        for _ in range(2):
            pd0 = ps_dx.tile([PW, 512], F32, tag="dxc")
            nc.vector.memset(pd0, 0.0)

        def half_slice(tile_, p0, cnt, hf):
            return tile_[p0:p0 + cnt, :].rearrange(
                "p (r h) -> p r h", h=NH)[:, :, hf]

        def mask_phase(img):
            S = img * HNH
            # vertical 3-max (rows clamped at image edge); row step in rh = NH
            nc.vector.tensor_tensor(
                out=vm[:, NH:], in0=am[:, S + NH:S + HNH],
                in1=am[:, S:S + HNH - NH], op=AL.max)
            nc.vector.tensor_copy(out=vm[:, 0:NH], in_=am[:, S:S + NH])
            nc.vector.tensor_tensor(
                out=vm2[:, 0:HNH - NH], in0=vm[:, 0:HNH - NH],
                in1=am[:, S + NH:S + HNH], op=AL.max)
            nc.vector.tensor_copy(out=vm2[:, HNH - NH:], in_=vm[:, HNH - NH:])

            # horizontal 3-max: PE shift-permutation matmuls (bf16) + seams
            pse = ps_h.tile([PW, HNH], F32, tag="psh")
            nc.tensor.matmul(out=pse, lhsT=se_sb, rhs=vm2,
                             start=True, stop=True)
            psw = ps_h.tile([PW, HNH], F32, tag="psh")
            nc.tensor.matmul(out=psw, lhsT=sw_sb, rhs=vm2,
                             start=True, stop=True)
            nc.vector.tensor_tensor(out=m3, in0=vm2, in1=pse, op=AL.max)
            nc.vector.tensor_tensor(out=m3, in0=m3, in1=psw, op=AL.max)

            # seam fixes between the two w-halves (zero elsewhere is
            # harmless for the >0.1 threshold)
            nc.sync.dma_start(out=seam[127:128, 0:H],
                              in_=half_slice(vm2, 0, 1, 1))
            nc.vector.tensor_tensor(
                out=half_slice(m3, 96, 32, 0), in0=half_slice(m3, 96, 32, 0),
                in1=seam[96:128, 0:H], op=AL.max)
            nc.sync.dma_start(out=seam[0:1, H:2 * H],
                              in_=half_slice(vm2, 127, 1, 0))
            nc.vector.tensor_tensor(
                out=half_slice(m3, 0, 32, 1), in0=half_slice(m3, 0, 32, 1),
                in1=seam[0:32, H:2 * H], op=AL.max)

            # life = (m3 > thr) * pre_life
            nc.vector.scalar_tensor_tensor(
                out=life, in0=m3, scalar=ALIVE_THR,
                in1=plx_sb[:, S:S + HNH], op0=AL.is_gt, op1=AL.mult)

            # x_new = x_mid * life, in 4 chunks alternating DVE/GPSIMD
            CH = HNH // 4
            for chk in range(4):
                rr = S + chk * CH
                xns = xns_pool.tile([PW, CH, C], BF16)
                eng = nc.vector if chk % 2 == 0 else nc.gpsimd
                eng.tensor_tensor(
                    out=xns, in0=xmid[:, rr:rr + CH, :],
                    in1=life[:, chk * CH:(chk + 1) * CH, None].to_broadcast(
                        [PW, CH, C]),
                    op=AL.mult)
                nc.scalar.dma_start(out=xno.ap()[:, rr:rr + CH, :], in_=xns)

        # ================= main loop =================
        for tt in range(NT):
            img, t_in = divmod(tt, H // TR)
            a = t_in * TR
            rh_t = tt * TR * NH

            # x in conv layout: partitions (dy,c), shifted copy on 48..95
            xc2 = xc_pool.tile([96, TR, Wp], BF16)
            base = (img * C * Hp + a) * Wp
            for s in range(2):
                src = bass.AP(
                    tensor=xch.ap().tensor, offset=base + s,
                    ap=[[Wp, 3], [Hp * Wp, C], [Wp, TR], [1, Wp]])
                nc.sync.dma_start(out=xc2[48 * s:48 * s + 48], in_=src)

            # x in pixel layout for the tail
            xp = xp_pool.tile([PW, TR * NH, C], BF16)
            nc.sync.dma_start(out=xp, in_=xpx_d.ap()[:, rh_t:rh_t + TR * NH, :])

            # ---- fc0: 2 streamed matmuls per row-pair group ----
            pshs = [None] * 8
            hsbs = [None] * 8
            for q in range(2):
                for g in range(4):
                    pp = q * 4 + g
                    psh = ps_h.tile([HID, 512], F32, tag="psh")
                    pshs[pp] = psh
                    nc.tensor.matmul(
                        out=psh, lhsT=b01_sb,
                        rhs=xc2[0:96, 2 * pp:2 * pp + 2, 0:W],
                        start=True, stop=False, skip_group_check=True)
                for g in range(4):
                    pp = q * 4 + g
                    nc.tensor.matmul(
                        out=pshs[pp], lhsT=b2_sb,
                        rhs=xc2[0:48, 2 * pp:2 * pp + 2, 2:2 + W],
                        start=False, stop=True, skip_group_check=True)
                for g in range(4):
                    pp = q * 4 + g
                    hsb = hs_pool.tile([HID, 512], BF16)
                    hsbs[pp] = hsb
                    if pp in (1, 3, 6):
                        nc.scalar.activation(out=hsb, in_=pshs[pp], func=Relu)
                    else:
                        nc.vector.tensor_scalar(
                            out=hsb, in0=pshs[pp], scalar1=0.0, scalar2=None,
                            op0=AL.max)

            # ---- fc1 (w1 stationary, col-group packed) + tail ----
            dxcm = dxc_pool.tile([PW, 2, 512], BF16)
            for q in range(2):
                pd = ps_dx.tile([PW, 512], F32, tag="dxc")
                for g in range(4):
                    pp = q * 4 + g
                    nc.tensor.matmul(
                        out=pd[32 * g:32 * g + 16, :], lhsT=w1_sb,
                        rhs=hsbs[pp], start=True, stop=True,
                        skip_group_check=True, tile_position=(0, 32 * g))
                nc.vector.tensor_copy(out=dxcm[:, q, :], in_=pd)

                # xbar transpose: [16c, 512px] slabs -> [128w, 4rh, 16c]
                dxt = dxt_pool.tile([PW, TR, C], BF16)
                for g in range(4):
                    nc.sync.dma_start(
                        out=dxt[:, 4 * g:4 * g + 4, :],
                        in_=dxcm[32 * g:32 * g + 16, q, :], transpose=True)

                rh0 = rh_t + TR * q
                dxm = dxm_pool.tile([PW, TR, C], F32)
                nc.gpsimd.tensor_tensor(
                    out=dxm, in0=dxt,
                    in1=um_sb[:, rh0:rh0 + TR, None].to_broadcast([PW, TR, C]),
                    op=AL.mult)
                nc.gpsimd.tensor_tensor(
                    out=xmid[:, rh0:rh0 + TR, :],
                    in0=xp[:, TR * q:TR * q + TR, :], in1=dxm, op=AL.add)
                nc.gpsimd.tensor_copy(
                    out=am[:, rh0:rh0 + TR], in_=xmid[:, rh0:rh0 + TR, 3])

            nc.scalar.dma_start(out=dxo.ap()[tt], in_=dxcm)

            if a + TR == H:
                mask_phase(img)


# ---------------------------------------------------------------------------
# host side
# ---------------------------------------------------------------------------

def _sobel():
    kx = np.outer([1.0, 2.0, 1.0], [-1.0, 0.0, 1.0]) / 8.0
    ky = kx.T
    return kx, ky


def make_weights(w0, w1):
    """Fold sobel taps into fc0 -> b01 [96,128] (taps dx=0,1), b2 [48,128]."""
    kx, ky = _sobel()
    w0 = np.asarray(w0, np.float32)          # [48, 128]
    W0x = w0[0::3]                           # [16, 128]
    W0gx = w0[1::3]
    W0gy = w0[2::3]
    Bw = np.zeros((3, 48, HID), np.float32)
    for dy in range(3):
        for dxi in range(3):
            m = kx[dy, dxi] * W0gx + ky[dy, dxi] * W0gy
            if dy == 1 and dxi == 1:
                m = m + W0x
            Bw[dxi, dy * C:(dy + 1) * C, :] = m
    b01 = np.concatenate([Bw[0], Bw[1]], axis=0).astype(BF16_NP)
    b2 = Bw[2].astype(BF16_NP)
    return b01, b2, np.asarray(w1, BF16_NP)


def _maxpool3(a):
    """3x3 max pool, stride 1, -inf padding. a: [NI, H, W]."""
    p = np.pad(a, ((0, 0), (1, 1), (1, 1)), constant_values=-np.inf)
    v = np.maximum(np.maximum(p[:, :-2, :], p[:, 1:-1, :]), p[:, 2:, :])
    return np.maximum(np.maximum(v[:, :, :-2], v[:, :, 1:-1]), v[:, :, 2:])


def _to_rh(a, NI, H, NH):
    """[NI, H, W] -> [PW, NI*H*NH] pixel-slab layout."""
    return np.ascontiguousarray(
        a.reshape(NI, H, NH, PW).transpose(3, 0, 1, 2)).reshape(PW, NI * H * NH)


def host_inputs(x_core, ur_core, b01, b2, w1, H, W):
    """Build the per-core input map from [NI,H,W,C] x and [NI,H,W,1] rand."""
    NI = x_core.shape[0]
    NH = W // PW
    Hp, Wp = H + 2, W + 2
    NRH = NI * H * NH

    xch = np.zeros((NI, C, Hp, Wp), BF16_NP)
    xch[:, :, 1:H + 1, 1:W + 1] = x_core.transpose(0, 3, 1, 2)
    xch = np.concatenate(
        [xch.reshape(NI * C * Hp, Wp), np.zeros((1, Wp), BF16_NP)], axis=0)

    x_px = np.ascontiguousarray(
        x_core.reshape(NI, H, NH, PW, C).transpose(3, 0, 1, 2, 4)
    ).reshape(PW, NRH, C).astype(BF16_NP)

    um = (ur_core[..., 0] <= FIRE_RATE).astype(np.float32)
    um_p = _to_rh(um, NI, H, NH).astype(BF16_NP)

    pre = (_maxpool3(x_core[..., 3]) > ALIVE_THR).astype(np.float32)
    plx_p = _to_rh(pre, NI, H, NH).astype(BF16_NP)

    return {
        "xch": xch,
        "xpx": x_px,
        "um": um_p,
        "plx": plx_p,
        "b01": b01,
        "b2": b2,
        "w1d": w1,
        "SEd": np.eye(PW, k=-1, dtype=np.float32).astype(BF16_NP),
        "SWd": np.eye(PW, k=1, dtype=np.float32).astype(BF16_NP),
    }


def unpack_xno(dev, NI, H, W):
    """[PW, NRH, C] device layout -> [NI, H, W, C] float32."""
    NH = W // PW
    return np.ascontiguousarray(
        dev.astype(np.float32).reshape(PW, NI, H, NH, C).transpose(1, 2, 3, 0, 4)
    ).reshape(NI, H, W, C)


def unpack_dxo(dev, NI, H, W, TR=16):
    """[NT, PW, 1024] channel-major packed dx -> [NI, H, W, C] float32."""
    NTI = H // TR                 # tiles per image
    d = dev.astype(np.float32).reshape(NI, NTI, 4, 32, 2, 2, 2, PW)
    # [img, t, g, cc, q, r2, h, w] -> [img, t, q, g, r2, h, w, c]
    d = d[:, :, :, :C].transpose(0, 1, 4, 2, 5, 6, 7, 3)
    return np.ascontiguousarray(d).reshape(NI, H, W, C)


@functools.lru_cache(maxsize=2)
def _cached_program(NI, H, W, TR):
    return build_program(NI, H, W, TR=TR)


def kernel(x, update_rand, w0, w1):
    x = np.asarray(x, np.float32)
    update_rand = np.asarray(update_rand, np.float32)
    B, H, W, _ = x.shape
    NI = B // N_CORES
    b01, b2, w1f = make_weights(w0, w1)

    nc = _cached_program(NI, H, W, 16)
    in_maps = [
        host_inputs(x[i * NI:(i + 1) * NI], update_rand[i * NI:(i + 1) * NI],
                    b01, b2, w1f, H, W)
        for i in range(N_CORES)
    ]
    res = run_bass_kernel_spmd(nc, in_maps, core_ids=list(range(N_CORES)))
    global LAST_RESULTS
    LAST_RESULTS = res
    x_new = np.concatenate(
        [unpack_xno(r["xno"], NI, H, W) for r in res.results], axis=0)
    dx = np.concatenate(
        [unpack_dxo(r["dxo"], NI, H, W) for r in res.results], axis=0)
    return x_new, dx


# revision 6
# speedup vs baseline: 1.5880x; 1.5880x over previous
"""Trainium2 Bass kernel for nn_CAModel (neural cellular automaton step).

Computation (per image, fp32):
  pre_life = maxpool3x3(x[...,3]) > 0.1        (HOST: exact fp32)
  gx, gy   = depthwise 3x3 sobel convs of x
  perc     = interleave([x, gx, gy])            # [H,W,48]
  h        = relu(perc @ w0)                    # [H,W,128]
  dx       = h @ w1                             # [H,W,16]
  x_mid    = x + dx * (update_rand <= 0.5)      (update mask from HOST)
  life     = pre_life & (maxpool3x3(x_mid[...,3]) > 0.1)
  x_new    = x_mid * life
  returns (x_new, dx)

Mapping: 8 NeuronCores, data-parallel over batch (2 images/core).

Device pipeline per 16-row tile (2 psum-halves q of 4 row-pair groups):
  fc0: sobel folded into weights; two K-stacked matmuls per row-pair
       (taps dx=0,1 stacked on 96 partitions, tap dx=2 separate) instead
       of three -> 2 streamed columns/pixel on the PE.
  relu: split across ACT and DVE engines (psum -> bf16 sbuf).
  fc1: w1-stationary (16-col weight loads), 4 row-pair groups packed
       into one PSUM tile at col-group offsets 0/32/64/96.
  dx transpose: channel-major [16,512] slabs -> pixel-major via HWDGE
       xbar DMA transpose (bf16).
  tail (gpsimd): x_mid = x + dx*um; alpha extract; per-image maxpool
       (vertical on DVE, horizontal via bf16 shift matmuls on PE),
       life mult, x_new store.  All I/O in bf16.
"""

import functools
import os
import sys

import numpy as np

_TRN_REPO = os.environ.get("TRN_RL_REPO", "/opt/trn_rl_repo")
if _TRN_REPO not in sys.path:
    sys.path.insert(0, _TRN_REPO)

import concourse.bass as bass
import concourse.bacc as bacc
import concourse.tile as tile
from concourse import mybir
from concourse.bass_utils import run_bass_kernel_spmd

F32 = mybir.dt.float32
BF16 = mybir.dt.bfloat16
BF16_NP = mybir.dt.np(mybir.dt.bfloat16)

C = 16          # channels
HID = 128       # hidden dim
PW = 128        # partitions used as w-position within a half
N_CORES = 8
FIRE_RATE = 0.5
ALIVE_THR = 0.1

LAST_RESULTS = None  # BassKernelResults of the most recent kernel() call


# ---------------------------------------------------------------------------
# device program
# ---------------------------------------------------------------------------

def build_program(NI, H, W, TR=16):
    """Build the Bass program for one core processing NI images of HxW."""
    NH = W // PW                  # halves per row (2)
    assert W % PW == 0 and H % TR == 0 and TR == 16 and NH == 2
    Hp, Wp = H + 2, W + 2
    NRH = NI * H * NH             # total (img,row,half) count
    NT = NI * H // TR             # tiles

    nc = bacc.Bacc(trn_type="TRN2")

    xch = nc.dram_tensor("xch", [NI * C * Hp + 1, Wp], BF16, kind="ExternalInput")
    xpx_d = nc.dram_tensor("xpx", [PW, NRH, C], BF16, kind="ExternalInput")
    um_d = nc.dram_tensor("um", [PW, NRH], BF16, kind="ExternalInput")
    plx_d = nc.dram_tensor("plx", [PW, NRH], BF16, kind="ExternalInput")
    b01_d = nc.dram_tensor("b01", [96, HID], BF16, kind="ExternalInput")
    b2_d = nc.dram_tensor("b2", [48, HID], BF16, kind="ExternalInput")
    w1_d = nc.dram_tensor("w1d", [HID, C], BF16, kind="ExternalInput")
    se_d = nc.dram_tensor("SEd", [PW, PW], BF16, kind="ExternalInput")
    sw_d = nc.dram_tensor("SWd", [PW, PW], BF16, kind="ExternalInput")
    dxo = nc.dram_tensor("dxo", [NT // 2, PW, 4 * 512], BF16,
                         kind="ExternalOutput")
    xno = nc.dram_tensor("xno", [PW, NRH, C], BF16, kind="ExternalOutput")

    with tile.TileContext(nc) as tc:
        _emit(tc, locals())
    nc.compile()
    return nc


def _emit(tc, t):
    nc = tc.nc
    NI, H, W, TR = t["NI"], t["H"], t["W"], t["TR"]
    NH, Hp, Wp, NRH, NT = t["NH"], t["Hp"], t["Wp"], t["NRH"], t["NT"]
    xch, xpx_d, um_d, plx_d = t["xch"], t["xpx_d"], t["um_d"], t["plx_d"]
    b01_d, b2_d, w1_d, se_d, sw_d = (
        t["b01_d"], t["b2_d"], t["w1_d"], t["se_d"], t["sw_d"])
    dxo, xno = t["dxo"], t["xno"]
    HNH = H * NH                  # per-image rh span (512)
    AL = mybir.AluOpType
    Relu = mybir.ActivationFunctionType.Relu

    from contextlib import ExitStack
    ctx = ExitStack()
    with ctx:
        singles = ctx.enter_context(tc.tile_pool(name="singles", bufs=1))
        xc_pool = ctx.enter_context(tc.tile_pool(name="xc", bufs=2))
        hs_pool = ctx.enter_context(tc.tile_pool(name="hs", bufs=8))
        xp_pool = ctx.enter_context(tc.tile_pool(name="xp", bufs=2))
        dxc_pool = ctx.enter_context(tc.tile_pool(name="dxc", bufs=2))
        dxt_pool = ctx.enter_context(tc.tile_pool(name="dxt", bufs=2))
        dxm_pool = ctx.enter_context(tc.tile_pool(name="dxm", bufs=2))
        xns_pool = ctx.enter_context(tc.tile_pool(name="xns", bufs=2))
        ps_h = ctx.enter_context(tc.tile_pool(name="ps_h", bufs=5, space="PSUM"))
        ps_dx = ctx.enter_context(tc.tile_pool(name="ps_dx", bufs=2, space="PSUM"))
        ps_scr = ctx.enter_context(tc.tile_pool(name="ps_scr", bufs=1, space="PSUM"))

        # ---- constants / weights / masks ----
        b01_sb = singles.tile([96, HID], BF16)
        nc.sync.dma_start(out=b01_sb, in_=b01_d.ap())
        b2_sb = singles.tile([48, HID], BF16)
        nc.sync.dma_start(out=b2_sb, in_=b2_d.ap())
        w1_sb = singles.tile([HID, C], BF16)
        nc.sync.dma_start(out=w1_sb, in_=w1_d.ap())
        se_sb = singles.tile([PW, PW], BF16)
        nc.sync.dma_start(out=se_sb, in_=se_d.ap())
        sw_sb = singles.tile([PW, PW], BF16)
        nc.sync.dma_start(out=sw_sb, in_=sw_d.ap())
        um_sb = singles.tile([PW, NRH], BF16)
        nc.sync.dma_start(out=um_sb, in_=um_d.ap())
        plx_sb = singles.tile([PW, NRH], BF16)
        nc.sync.dma_start(out=plx_sb, in_=plx_d.ap())

        # ---- residents / mask scratch ----
        xmid = singles.tile([PW, NRH, C], BF16)
        am = singles.tile([PW, NRH], BF16)
        vm = singles.tile([PW, HNH], BF16)
        vm2 = singles.tile([PW, HNH], BF16)
        m3 = singles.tile([PW, HNH], BF16)
        life = singles.tile([PW, HNH], BF16)
        seam = singles.tile([PW, 2 * H], BF16)
        nc.vector.memset(seam, 0.0)

        # ---- PE pre-sync dummies (touch each DMA'd matmul operand once) ----
        scr = ps_scr.tile([PW, 2], F32, tag="scr")
        nc.tensor.matmul(out=scr, lhsT=b01_sb, rhs=b01_sb[:, 0:2],
                         start=True, stop=True)
        nc.tensor.matmul(out=scr, lhsT=b2_sb, rhs=b2_sb[:, 0:2],
                         start=True, stop=True)
        scr2f = ps_scr.tile([PW, 2], F32, tag="scr")
        nc.tensor.matmul(out=scr2f[0:C, :], lhsT=w1_sb, rhs=w1_sb[:, 0:2],
                         start=True, stop=True)
        nc.tensor.matmul(out=scr, lhsT=se_sb, rhs=se_sb[:, 0:2],
                         start=True, stop=True)
        nc.tensor.matmul(out=scr, lhsT=sw_sb, rhs=sw_sb[:, 0:2],
                         start=True, stop=True)

        # ---- prime ps_dx garbage lanes so full-tile reads are defined ----
        for _ in range(2):
            pd0 = ps_dx.tile([PW, 512], F32, tag="dxc")
            nc.vector.memset(pd0, 0.0)

        def half_slice(tile_, p0, cnt, hf):
            return tile_[p0:p0 + cnt, :].rearrange(
                "p (r h) -> p r h", h=NH)[:, :, hf]

        def mask_phase(img):
            S = img * HNH
            # vertical 3-max (rows clamped at image edge); row step in rh = NH
            nc.vector.tensor_tensor(
                out=vm[:, NH:], in0=am[:, S + NH:S + HNH],
                in1=am[:, S:S + HNH - NH], op=AL.max)
            nc.vector.tensor_copy(out=vm[:, 0:NH], in_=am[:, S:S + NH])
            nc.vector.tensor_tensor(
                out=vm2[:, 0:HNH - NH], in0=vm[:, 0:HNH - NH],
                in1=am[:, S + NH:S + HNH], op=AL.max)
            nc.vector.tensor_copy(out=vm2[:, HNH - NH:], in_=vm[:, HNH - NH:])

            # horizontal 3-max: PE shift-permutation matmuls (bf16) + seams
            pse = ps_h.tile([PW, HNH], F32, tag="psh")
            nc.tensor.matmul(out=pse, lhsT=se_sb, rhs=vm2,
                             start=True, stop=True)
            psw = ps_h.tile([PW, HNH], F32, tag="psh")
            nc.tensor.matmul(out=psw, lhsT=sw_sb, rhs=vm2,
                             start=True, stop=True)
            nc.vector.tensor_tensor(out=m3, in0=vm2, in1=pse, op=AL.max)
            nc.vector.tensor_tensor(out=m3, in0=m3, in1=psw, op=AL.max)

            # seam fixes between the two w-halves (zero elsewhere is
            # harmless for the >0.1 threshold)
            nc.sync.dma_start(out=seam[127:128, 0:H],
                              in_=half_slice(vm2, 0, 1, 1))
            nc.vector.tensor_tensor(
                out=half_slice(m3, 96, 32, 0), in0=half_slice(m3, 96, 32, 0),
                in1=seam[96:128, 0:H], op=AL.max)
            nc.sync.dma_start(out=seam[0:1, H:2 * H],
                              in_=half_slice(vm2, 127, 1, 0))
            nc.vector.tensor_tensor(
                out=half_slice(m3, 0, 32, 1), in0=half_slice(m3, 0, 32, 1),
                in1=seam[0:32, H:2 * H], op=AL.max)

            # life = (m3 > thr) * pre_life
            nc.vector.scalar_tensor_tensor(
                out=life, in0=m3, scalar=ALIVE_THR,
                in1=plx_sb[:, S:S + HNH], op0=AL.is_gt, op1=AL.mult)

            # x_new = x_mid * life, in 4 chunks alternating DVE/GPSIMD
            CH = HNH // 4
            for chk in range(4):
                rr = S + chk * CH
                xns = xns_pool.tile([PW, CH, C], BF16)
                eng = nc.vector if chk % 2 == 0 else nc.gpsimd
                eng.tensor_tensor(
                    out=xns, in0=xmid[:, rr:rr + CH, :],
                    in1=life[:, chk * CH:(chk + 1) * CH, None].to_broadcast(
                        [PW, CH, C]),
                    op=AL.mult)
                nc.scalar.dma_start(out=xno.ap()[:, rr:rr + CH, :], in_=xns)

        # ================= main loop (2-tile super-tiles) =================
        NST = NT // 2
        for st in range(NST):
            # x in pixel layout for the tail, 2 tiles worth
            rh_s = st * 2 * TR * NH
            xp = xp_pool.tile([PW, 2 * TR * NH, C], BF16)
            nc.sync.dma_start(
                out=xp, in_=xpx_d.ap()[:, rh_s:rh_s + 2 * TR * NH, :])

            dxcm = dxc_pool.tile([PW, 2, 2, 512], BF16)  # [p, ti, q, f]
            ncp = 0
            for ti in range(2):
                tt = 2 * st + ti
                img, t_in = divmod(tt, H // TR)
                a = t_in * TR

                # x in conv layout: partitions (dy,c), shifted copy on 48..95
                xc2 = xc_pool.tile([96, TR, Wp], BF16)
                base = (img * C * Hp + a) * Wp
                for s in range(2):
                    src = bass.AP(
                        tensor=xch.ap().tensor, offset=base + s,
                        ap=[[Wp, 3], [Hp * Wp, C], [Wp, TR], [1, Wp]])
                    nc.sync.dma_start(out=xc2[48 * s:48 * s + 48], in_=src)

                # ---- fc0: 2 streamed matmuls per row-pair group ----
                pshs = [None] * 8
                hsbs = [None] * 8
                for q in range(2):
                    for g in range(4):
                        pp = q * 4 + g
                        psh = ps_h.tile([HID, 512], F32, tag="psh")
                        pshs[pp] = psh
                        nc.tensor.matmul(
                            out=psh, lhsT=b01_sb,
                            rhs=xc2[0:96, 2 * pp:2 * pp + 2, 0:W],
                            start=True, stop=False, skip_group_check=True)
                    for g in range(4):
                        pp = q * 4 + g
                        nc.tensor.matmul(
                            out=pshs[pp], lhsT=b2_sb,
                            rhs=xc2[0:48, 2 * pp:2 * pp + 2, 2:2 + W],
                            start=False, stop=True, skip_group_check=True)
                    for g in range(4):
                        pp = q * 4 + g
                        hsb = hs_pool.tile([HID, 512], BF16)
                        hsbs[pp] = hsb
                        if pp % 2 == 0:
                            nc.scalar.activation(
                                out=hsb, in_=pshs[pp], func=Relu)
                        else:
                            nc.vector.tensor_scalar(
                                out=hsb, in0=pshs[pp], scalar1=0.0,
                                scalar2=None, op0=AL.max)

                # ---- fc1: w1 stationary, 4 col-groups run concurrently ----
                for q in range(2):
                    pd = ps_dx.tile([PW, 512], F32, tag="dxc")
                    for g in range(4):
                        pp = q * 4 + g
                        nc.tensor.matmul(
                            out=pd[32 * g:32 * g + 16, :], lhsT=w1_sb,
                            rhs=hsbs[pp], start=True, stop=True,
                            skip_group_check=True, tile_position=(0, 32 * g))
                    if ncp % 2 == 0:
                        nc.vector.tensor_copy(out=dxcm[:, ti, q, :], in_=pd)
                    else:
                        nc.scalar.copy(out=dxcm[:, ti, q, :], in_=pd)
                    ncp += 1

            # one xbar transpose for the whole super-tile:
            # [128(g,cc), 2048] -> [128w, 16(ti,q,r2,h), 128(g,cc)]
            dxt = dxt_pool.tile([PW, 16, PW], BF16)
            nc.sync.dma_start(out=dxt, in_=dxcm, transpose=True)
            nc.sync.dma_start(out=dxo.ap()[st], in_=dxcm)

            # ---- tail: x_mid = x + dx*um, alpha extract ----
            for ti in range(2):
                for q in range(2):
                    j0 = ti * 8 + q * 4
                    rh0 = rh_s + (ti * 2 + q) * TR
                    dxv = dxt[:, j0:j0 + 4, :].rearrange(
                        "p j (g cc) -> p g j cc", g=4)[:, :, :, 0:C]
                    umv = um_sb[:, rh0:rh0 + TR].rearrange(
                        "p (g j) -> p g j", g=4)[:, :, :, None].to_broadcast(
                        [PW, 4, 4, C])
                    dxm = dxm_pool.tile([PW, TR, C], F32)
                    nc.gpsimd.tensor_tensor(
                        out=dxm.rearrange("p (g j) c -> p g j c", g=4),
                        in0=dxv, in1=umv, op=AL.mult)
                    lq = (ti * 2 + q) * TR
                    nc.gpsimd.tensor_tensor(
                        out=xmid[:, rh0:rh0 + TR, :],
                        in0=xp[:, lq:lq + TR, :], in1=dxm, op=AL.add)
                    nc.vector.tensor_copy(
                        out=am[:, rh0:rh0 + TR], in_=xmid[:, rh0:rh0 + TR, 3])

            if (2 * st + 1) % (H // TR) == H // TR - 1:
                mask_phase((2 * st + 1) // (H // TR))


# ---------------------------------------------------------------------------
# host side
# ---------------------------------------------------------------------------

def _sobel():
    kx = np.outer([1.0, 2.0, 1.0], [-1.0, 0.0, 1.0]) / 8.0
    ky = kx.T
    return kx, ky


def make_weights(w0, w1):
    """Fold sobel taps into fc0 -> b01 [96,128] (taps dx=0,1), b2 [48,128]."""
    kx, ky = _sobel()
    w0 = np.asarray(w0, np.float32)          # [48, 128]
    W0x = w0[0::3]                           # [16, 128]
    W0gx = w0[1::3]
    W0gy = w0[2::3]
    Bw = np.zeros((3, 48, HID), np.float32)
    for dy in range(3):
        for dxi in range(3):
            m = kx[dy, dxi] * W0gx + ky[dy, dxi] * W0gy
            if dy == 1 and dxi == 1:
                m = m + W0x
            Bw[dxi, dy * C:(dy + 1) * C, :] = m
    b01 = np.concatenate([Bw[0], Bw[1]], axis=0).astype(BF16_NP)
    b2 = Bw[2].astype(BF16_NP)
    return b01, b2, np.asarray(w1, BF16_NP)


def _maxpool3(a):
    """3x3 max pool, stride 1, -inf padding. a: [NI, H, W]."""
    p = np.pad(a, ((0, 0), (1, 1), (1, 1)), constant_values=-np.inf)
    v = np.maximum(np.maximum(p[:, :-2, :], p[:, 1:-1, :]), p[:, 2:, :])
    return np.maximum(np.maximum(v[:, :, :-2], v[:, :, 1:-1]), v[:, :, 2:])


def _to_rh(a, NI, H, NH):
    """[NI, H, W] -> [PW, NI*H*NH] pixel-slab layout."""
    return np.ascontiguousarray(
        a.reshape(NI, H, NH, PW).transpose(3, 0, 1, 2)).reshape(PW, NI * H * NH)


def host_inputs(x_core, ur_core, b01, b2, w1, H, W):
    """Build the per-core input map from [NI,H,W,C] x and [NI,H,W,1] rand."""
    NI = x_core.shape[0]
    NH = W // PW
    Hp, Wp = H + 2, W + 2
    NRH = NI * H * NH

    xch = np.zeros((NI, C, Hp, Wp), BF16_NP)
    xch[:, :, 1:H + 1, 1:W + 1] = x_core.transpose(0, 3, 1, 2)
    xch = np.concatenate(
        [xch.reshape(NI * C * Hp, Wp), np.zeros((1, Wp), BF16_NP)], axis=0)

    x_px = np.ascontiguousarray(
        x_core.reshape(NI, H, NH, PW, C).transpose(3, 0, 1, 2, 4)
    ).reshape(PW, NRH, C).astype(BF16_NP)

    um = (ur_core[..., 0] <= FIRE_RATE).astype(np.float32)
    um_p = _to_rh(um, NI, H, NH).astype(BF16_NP)

    pre = (_maxpool3(x_core[..., 3]) > ALIVE_THR).astype(np.float32)
    plx_p = _to_rh(pre, NI, H, NH).astype(BF16_NP)

    return {
        "xch": xch,
        "xpx": x_px,
        "um": um_p,
        "plx": plx_p,
        "b01": b01,
        "b2": b2,
        "w1d": w1,
        "SEd": np.eye(PW, k=-1, dtype=np.float32).astype(BF16_NP),
        "SWd": np.eye(PW, k=1, dtype=np.float32).astype(BF16_NP),
    }


def unpack_xno(dev, NI, H, W):
    """[PW, NRH, C] device layout -> [NI, H, W, C] float32."""
    NH = W // PW
    return np.ascontiguousarray(
        dev.astype(np.float32).reshape(PW, NI, H, NH, C).transpose(1, 2, 3, 0, 4)
    ).reshape(NI, H, W, C)


def unpack_dxo(dev, NI, H, W, TR=16):
    """[NST, PW, 2048] channel-major packed dx -> [NI, H, W, C] float32."""
    NSI = H // TR // 2            # super-tiles per image
    d = dev.astype(np.float32).reshape(NI, NSI, 4, 32, 2, 2, 2, 2, PW)
    # [img, st, g, cc, ti, q, r2, h, w] -> [img, st, ti, q, g, r2, h, w, c]
    d = d[:, :, :, :C].transpose(0, 1, 4, 5, 2, 6, 7, 8, 3)
    return np.ascontiguousarray(d).reshape(NI, H, W, C)


@functools.lru_cache(maxsize=2)
def _cached_program(NI, H, W, TR):
    return build_program(NI, H, W, TR=TR)


def kernel(x, update_rand, w0, w1):
    x = np.asarray(x, np.float32)
    update_rand = np.asarray(update_rand, np.float32)
    B, H, W, _ = x.shape
    NI = B // N_CORES
    b01, b2, w1f = make_weights(w0, w1)

    nc = _cached_program(NI, H, W, 16)
    in_maps = [
        host_inputs(x[i * NI:(i + 1) * NI], update_rand[i * NI:(i + 1) * NI],
                    b01, b2, w1f, H, W)
        for i in range(N_CORES)
    ]
    res = run_bass_kernel_spmd(nc, in_maps, core_ids=list(range(N_CORES)))
    global LAST_RESULTS
    LAST_RESULTS = res
    x_new = np.concatenate(
        [unpack_xno(r["xno"], NI, H, W) for r in res.results], axis=0)
    dx = np.concatenate(
        [unpack_dxo(r["dxo"], NI, H, W) for r in res.results], axis=0)
    return x_new, dx


# revision 12
# speedup vs baseline: 1.6555x; 1.0425x over previous
"""Trainium2 Bass kernel for nn_CAModel (neural cellular automaton step).

Computation (per image, fp32):
  pre_life = maxpool3x3(x[...,3]) > 0.1        (HOST: exact fp32)
  gx, gy   = depthwise 3x3 sobel convs of x
  perc     = interleave([x, gx, gy])            # [H,W,48]
  h        = relu(perc @ w0)                    # [H,W,128]
  dx       = h @ w1                             # [H,W,16]
  x_mid    = x + dx * (update_rand <= 0.5)      (update mask from HOST)
  life     = pre_life & (maxpool3x3(x_mid[...,3]) > 0.1)
  x_new    = x_mid * life
  returns (x_new, dx)

Mapping: 8 NeuronCores, data-parallel over batch (2 images/core).

Device pipeline per 16-row tile (2 psum-halves q of 4 row-pair groups):
  fc0: sobel folded into weights; two K-stacked matmuls per row-pair
       (taps dx=0,1 stacked on 96 partitions, tap dx=2 separate) instead
       of three -> 2 streamed columns/pixel on the PE.
  relu: split across ACT and DVE engines (psum -> bf16 sbuf).
  fc1: w1-stationary (16-col weight loads), 4 row-pair groups packed
       into one PSUM tile at col-group offsets 0/32/64/96.
  dx transpose: channel-major [16,512] slabs -> pixel-major via HWDGE
       xbar DMA transpose (bf16).
  tail (gpsimd): x_mid = x + dx*um; alpha extract; per-image maxpool
       (vertical on DVE, horizontal via bf16 shift matmuls on PE),
       life mult, x_new store.  All I/O in bf16.
"""

import functools
import os
import sys

import numpy as np

_TRN_REPO = os.environ.get("TRN_RL_REPO", "/opt/trn_rl_repo")
if _TRN_REPO not in sys.path:
    sys.path.insert(0, _TRN_REPO)

import concourse.bass as bass
import concourse.bacc as bacc
import concourse.tile as tile
from concourse import mybir
from concourse.bass_utils import run_bass_kernel_spmd

F32 = mybir.dt.float32
BF16 = mybir.dt.bfloat16
BF16_NP = mybir.dt.np(mybir.dt.bfloat16)

C = 16          # channels
HID = 128       # hidden dim
PW = 128        # partitions used as w-position within a half
N_CORES = 8
FIRE_RATE = 0.5
ALIVE_THR = 0.1

LAST_RESULTS = None  # BassKernelResults of the most recent kernel() call


# ---------------------------------------------------------------------------
# device program
# ---------------------------------------------------------------------------

def build_program(NI, H, W, TR=16):
    """Build the Bass program for one core processing NI images of HxW."""
    NH = W // PW                  # halves per row (2)
    assert W % PW == 0 and H % TR == 0 and TR == 16 and NH == 2
    Hp, Wp = H + 2, W + 2
    NRH = NI * H * NH             # total (img,row,half) count
    NT = NI * H // TR             # tiles

    nc = bacc.Bacc(trn_type="TRN2")

    xch = nc.dram_tensor("xch", [NI * C * Hp + 1, Wp], BF16, kind="ExternalInput")
    xpx_d = nc.dram_tensor("xpx", [PW, NRH, C], BF16, kind="ExternalInput")
    um_d = nc.dram_tensor("um", [PW, NRH], BF16, kind="ExternalInput")
    plx_d = nc.dram_tensor("plx", [PW, NRH], BF16, kind="ExternalInput")
    b01_d = nc.dram_tensor("b01", [96, HID], BF16, kind="ExternalInput")
    b2_d = nc.dram_tensor("b2", [48, HID], BF16, kind="ExternalInput")
    w1_d = nc.dram_tensor("w1d", [HID, C], BF16, kind="ExternalInput")
    se_d = nc.dram_tensor("SEd", [PW, PW], BF16, kind="ExternalInput")
    sw_d = nc.dram_tensor("SWd", [PW, PW], BF16, kind="ExternalInput")
    dxo = nc.dram_tensor("dxo", [NT // 2, PW, 4 * 512], BF16,
                         kind="ExternalOutput")
    xno = nc.dram_tensor("xno", [PW, NRH, C], BF16, kind="ExternalOutput")

    with tile.TileContext(nc) as tc:
        _emit(tc, locals())
    nc.compile()
    return nc


def _emit(tc, t):
    nc = tc.nc
    NI, H, W, TR = t["NI"], t["H"], t["W"], t["TR"]
    NH, Hp, Wp, NRH, NT = t["NH"], t["Hp"], t["Wp"], t["NRH"], t["NT"]
    xch, xpx_d, um_d, plx_d = t["xch"], t["xpx_d"], t["um_d"], t["plx_d"]
    b01_d, b2_d, w1_d, se_d, sw_d = (
        t["b01_d"], t["b2_d"], t["w1_d"], t["se_d"], t["sw_d"])
    dxo, xno = t["dxo"], t["xno"]
    HNH = H * NH                  # per-image rh span (512)
    AL = mybir.AluOpType
    Relu = mybir.ActivationFunctionType.Relu

    from contextlib import ExitStack
    ctx = ExitStack()
    with ctx:
        singles = ctx.enter_context(tc.tile_pool(name="singles", bufs=1))
        xc_pool = ctx.enter_context(tc.tile_pool(name="xc", bufs=4))
        hs_pool = ctx.enter_context(tc.tile_pool(name="hs", bufs=8))
        xp_pool = ctx.enter_context(tc.tile_pool(name="xp", bufs=2))
        dxc_pool = ctx.enter_context(tc.tile_pool(name="dxc", bufs=2))
        dxt_pool = ctx.enter_context(tc.tile_pool(name="dxt", bufs=2))
        dxm_pool = ctx.enter_context(tc.tile_pool(name="dxm", bufs=2))
        xns_pool = ctx.enter_context(tc.tile_pool(name="xns", bufs=2))
        ps_h = ctx.enter_context(tc.tile_pool(name="ps_h", bufs=5, space="PSUM"))
        ps_dx = ctx.enter_context(tc.tile_pool(name="ps_dx", bufs=2, space="PSUM"))
        ps_scr = ctx.enter_context(tc.tile_pool(name="ps_scr", bufs=1, space="PSUM"))

        # ---- constants / weights / masks ----
        b01_sb = singles.tile([96, HID], BF16)
        nc.sync.dma_start(out=b01_sb, in_=b01_d.ap())
        b2_sb = singles.tile([48, HID], BF16)
        nc.sync.dma_start(out=b2_sb, in_=b2_d.ap())
        w1_sb = singles.tile([HID, C], BF16)
        nc.sync.dma_start(out=w1_sb, in_=w1_d.ap())
        se_sb = singles.tile([PW, PW], BF16)
        nc.sync.dma_start(out=se_sb, in_=se_d.ap())
        sw_sb = singles.tile([PW, PW], BF16)
        nc.sync.dma_start(out=sw_sb, in_=sw_d.ap())
        um_sb = singles.tile([PW, NRH], BF16)
        nc.sync.dma_start(out=um_sb, in_=um_d.ap())
        plx_sb = singles.tile([PW, NRH], BF16)
        nc.sync.dma_start(out=plx_sb, in_=plx_d.ap())

        # ---- residents / mask scratch ----
        xmid = singles.tile([PW, NRH, C], BF16)
        vm = singles.tile([PW, HNH], BF16)
        vm2 = singles.tile([PW, HNH], BF16)
        m3 = singles.tile([PW, HNH], BF16)
        life = singles.tile([PW, HNH], BF16)
        seam = singles.tile([PW, 2 * H], BF16)
        nc.vector.memset(seam, 0.0)

        # ---- PE pre-sync dummies (touch each DMA'd matmul operand once) ----
        scr = ps_scr.tile([PW, 2], F32, tag="scr")
        nc.tensor.matmul(out=scr, lhsT=b01_sb, rhs=b01_sb[:, 0:2],
                         start=True, stop=True)
        nc.tensor.matmul(out=scr, lhsT=b2_sb, rhs=b2_sb[:, 0:2],
                         start=True, stop=True)
        scr2f = ps_scr.tile([PW, 2], F32, tag="scr")
        nc.tensor.matmul(out=scr2f[0:C, :], lhsT=w1_sb, rhs=w1_sb[:, 0:2],
                         start=True, stop=True)
        nc.tensor.matmul(out=scr, lhsT=se_sb, rhs=se_sb[:, 0:2],
                         start=True, stop=True)
        nc.tensor.matmul(out=scr, lhsT=sw_sb, rhs=sw_sb[:, 0:2],
                         start=True, stop=True)

        # ---- prime ps_dx garbage lanes so full-tile reads are defined ----
        for _ in range(2):
            pd0 = ps_dx.tile([PW, 512], F32, tag="dxc")
            nc.vector.memset(pd0, 0.0)

        def half_slice(tile_, p0, cnt, hf):
            return tile_[p0:p0 + cnt, :].rearrange(
                "p (r h) -> p r h", h=NH)[:, :, hf]

        def mask_phase(img):
            S = img * HNH

            def A(a, b):  # alpha channel of x_mid, strided view
                return xmid[:, S + a:S + b, 3]

            # vertical 3-max (rows clamped at image edge); row step in rh = NH
            nc.vector.tensor_tensor(
                out=vm[:, NH:], in0=A(NH, HNH), in1=A(0, HNH - NH), op=AL.max)
            nc.vector.tensor_copy(out=vm[:, 0:NH], in_=A(0, NH))
            nc.vector.tensor_tensor(
                out=vm2[:, 0:HNH - NH], in0=vm[:, 0:HNH - NH],
                in1=A(NH, HNH), op=AL.max)
            nc.vector.tensor_copy(out=vm2[:, HNH - NH:], in_=vm[:, HNH - NH:])

            # horizontal 3-max: PE shift-permutation matmuls (bf16) + seams
            pse = ps_h.tile([PW, HNH], F32, tag="psh")
            nc.tensor.matmul(out=pse, lhsT=se_sb, rhs=vm2,
                             start=True, stop=True)
            psw = ps_h.tile([PW, HNH], F32, tag="psh")
            nc.tensor.matmul(out=psw, lhsT=sw_sb, rhs=vm2,
                             start=True, stop=True)
            nc.vector.tensor_tensor(out=m3, in0=vm2, in1=pse, op=AL.max)
            nc.vector.tensor_tensor(out=m3, in0=m3, in1=psw, op=AL.max)

            # seam fixes between the two w-halves (zero elsewhere is
            # harmless for the >0.1 threshold)
            nc.sync.dma_start(out=seam[127:128, 0:H],
                              in_=half_slice(vm2, 0, 1, 1))
            nc.vector.tensor_tensor(
                out=half_slice(m3, 96, 32, 0), in0=half_slice(m3, 96, 32, 0),
                in1=seam[96:128, 0:H], op=AL.max)
            nc.sync.dma_start(out=seam[0:1, H:2 * H],
                              in_=half_slice(vm2, 127, 1, 0))
            nc.vector.tensor_tensor(
                out=half_slice(m3, 0, 32, 1), in0=half_slice(m3, 0, 32, 1),
                in1=seam[0:32, H:2 * H], op=AL.max)

            # life = (m3 > thr) * pre_life
            nc.vector.scalar_tensor_tensor(
                out=life, in0=m3, scalar=ALIVE_THR,
                in1=plx_sb[:, S:S + HNH], op0=AL.is_gt, op1=AL.mult)

            # x_new = x_mid * life, in 4 chunks alternating DVE/GPSIMD
            CH = HNH // 4
            for chk in range(4):
                rr = S + chk * CH
                xns = xns_pool.tile([PW, CH, C], BF16)
                eng = nc.vector if chk % 2 == 0 else nc.gpsimd
                eng.tensor_tensor(
                    out=xns, in0=xmid[:, rr:rr + CH, :],
                    in1=life[:, chk * CH:(chk + 1) * CH, None].to_broadcast(
                        [PW, CH, C]),
                    op=AL.mult)
                nc.scalar.dma_start(out=xno.ap()[:, rr:rr + CH, :], in_=xns)

        # ================= main loop (2-tile super-tiles) =================
        NST = NT // 2

        def emit_loads(st):
            rh_s = st * 2 * TR * NH
            xp = xp_pool.tile([PW, 2 * TR * NH, C], BF16)
            nc.sync.dma_start(
                out=xp, in_=xpx_d.ap()[:, rh_s:rh_s + 2 * TR * NH, :])
            xcs = []
            for ti in range(2):
                tt = 2 * st + ti
                img, t_in = divmod(tt, H // TR)
                a = t_in * TR
                # x in conv layout: partitions (dy,c), shifted copy on 48..95
                xc2 = xc_pool.tile([96, TR, Wp], BF16)
                base = (img * C * Hp + a) * Wp
                for s in range(2):
                    src = bass.AP(
                        tensor=xch.ap().tensor, offset=base + s,
                        ap=[[Wp, 3], [Hp * Wp, C], [Wp, TR], [1, Wp]])
                    nc.sync.dma_start(out=xc2[48 * s:48 * s + 48], in_=src)
                xcs.append(xc2)
            return xp, xcs

        pending_mask = None
        ld = emit_loads(0)
        for st in range(NST):
            xp, xcs = ld
            rh_s = st * 2 * TR * NH
            dxcm = dxc_pool.tile([PW, 2, 2, 512], BF16)  # [p, ti, q, f]
            ncp = 0
            for ti in range(2):
                xc2 = xcs[ti]

                # ---- fc0: 2 streamed matmuls per row-pair group ----
                pshs = [None] * 8
                hsbs = [None] * 8
                for q in range(2):
                    for g in range(4):
                        pp = q * 4 + g
                        psh = ps_h.tile([HID, 512], F32, tag="psh")
                        pshs[pp] = psh
                        nc.tensor.matmul(
                            out=psh, lhsT=b01_sb,
                            rhs=xc2[0:96, 2 * pp:2 * pp + 2, 0:W],
                            start=True, stop=False, skip_group_check=True)
                    for g in range(4):
                        pp = q * 4 + g
                        nc.tensor.matmul(
                            out=pshs[pp], lhsT=b2_sb,
                            rhs=xc2[0:48, 2 * pp:2 * pp + 2, 2:2 + W],
                            start=False, stop=True, skip_group_check=True)
                    for g in range(4):
                        pp = q * 4 + g
                        hsb = hs_pool.tile([HID, 512], BF16)
                        hsbs[pp] = hsb
                        if pp % 2 == 0:
                            nc.scalar.activation(
                                out=hsb, in_=pshs[pp], func=Relu)
                        else:
                            nc.vector.tensor_scalar(
                                out=hsb, in0=pshs[pp], scalar1=0.0,
                                scalar2=None, op0=AL.max)

                # ---- fc1: w1 stationary, 4 col-groups run concurrently ----
                for q in range(2):
                    pd = ps_dx.tile([PW, 512], F32, tag="dxc")
                    for g in range(4):
                        pp = q * 4 + g
                        nc.tensor.matmul(
                            out=pd[32 * g:32 * g + 16, :], lhsT=w1_sb,
                            rhs=hsbs[pp], start=True, stop=True,
                            skip_group_check=True, tile_position=(0, 32 * g))
                    if ncp % 2 == 0:
                        nc.vector.tensor_copy(out=dxcm[:, ti, q, :], in_=pd)
                    else:
                        nc.scalar.copy(out=dxcm[:, ti, q, :], in_=pd)
                    ncp += 1

            # prefetch next super-tile's inputs ahead of the transpose on
            # the sync queue so they are never blocked behind it
            if st + 1 < NST:
                ld = emit_loads(st + 1)

            # one xbar transpose for the whole super-tile:
            # [128(g,cc), 2048] -> [128w, 16(ti,q,r2,h), 128(g,cc)]
            dxt = dxt_pool.tile([PW, 16, PW], BF16)
            nc.sync.dma_start(out=dxt, in_=dxcm, transpose=True)
            nc.sync.dma_start(out=dxo.ap()[st], in_=dxcm)

            # ---- tail: x_mid = x + dx*um, alpha extract ----
            for ti in range(2):
                for q in range(2):
                    j0 = ti * 8 + q * 4
                    rh0 = rh_s + (ti * 2 + q) * TR
                    dxv = dxt[:, j0:j0 + 4, :].rearrange(
                        "p j (g cc) -> p g j cc", g=4)[:, :, :, 0:C]
                    umv = um_sb[:, rh0:rh0 + TR].rearrange(
                        "p (g j) -> p g j", g=4)[:, :, :, None].to_broadcast(
                        [PW, 4, 4, C])
                    dxm = dxm_pool.tile([PW, TR, C], F32)
                    nc.gpsimd.tensor_tensor(
                        out=dxm.rearrange("p (g j) c -> p g j c", g=4),
                        in0=dxv, in1=umv, op=AL.mult)
                    lq = (ti * 2 + q) * TR
                    nc.gpsimd.tensor_tensor(
                        out=xmid[:, rh0:rh0 + TR, :],
                        in0=xp[:, lq:lq + TR, :], in1=dxm, op=AL.add)

            # mask phase deferred one super-tile so its inputs are ready
            # the moment its instructions reach the engine queues
            if pending_mask is not None:
                mask_phase(pending_mask)
                pending_mask = None
            if (2 * st + 1) % (H // TR) == H // TR - 1:
                pending_mask = (2 * st + 1) // (H // TR)
        if pending_mask is not None:
            mask_phase(pending_mask)


# ---------------------------------------------------------------------------
# host side
# ---------------------------------------------------------------------------

def _sobel():
    kx = np.outer([1.0, 2.0, 1.0], [-1.0, 0.0, 1.0]) / 8.0
    ky = kx.T
    return kx, ky


def make_weights(w0, w1):
    """Fold sobel taps into fc0 -> b01 [96,128] (taps dx=0,1), b2 [48,128]."""
    kx, ky = _sobel()
    w0 = np.asarray(w0, np.float32)          # [48, 128]
    W0x = w0[0::3]                           # [16, 128]
    W0gx = w0[1::3]
    W0gy = w0[2::3]
    Bw = np.zeros((3, 48, HID), np.float32)
    for dy in range(3):
        for dxi in range(3):
            m = kx[dy, dxi] * W0gx + ky[dy, dxi] * W0gy
            if dy == 1 and dxi == 1:
                m = m + W0x
            Bw[dxi, dy * C:(dy + 1) * C, :] = m
    b01 = np.concatenate([Bw[0], Bw[1]], axis=0).astype(BF16_NP)
    b2 = Bw[2].astype(BF16_NP)
    return b01, b2, np.asarray(w1, BF16_NP)


def _maxpool3(a):
    """3x3 max pool, stride 1, -inf padding. a: [NI, H, W]."""
    p = np.pad(a, ((0, 0), (1, 1), (1, 1)), constant_values=-np.inf)
    v = np.maximum(np.maximum(p[:, :-2, :], p[:, 1:-1, :]), p[:, 2:, :])
    return np.maximum(np.maximum(v[:, :, :-2], v[:, :, 1:-1]), v[:, :, 2:])


def _to_rh(a, NI, H, NH):
    """[NI, H, W] -> [PW, NI*H*NH] pixel-slab layout."""
    return np.ascontiguousarray(
        a.reshape(NI, H, NH, PW).transpose(3, 0, 1, 2)).reshape(PW, NI * H * NH)


def host_inputs(x_core, ur_core, b01, b2, w1, H, W):
    """Build the per-core input map from [NI,H,W,C] x and [NI,H,W,1] rand."""
    NI = x_core.shape[0]
    NH = W // PW
    Hp, Wp = H + 2, W + 2
    NRH = NI * H * NH

    xch = np.zeros((NI, C, Hp, Wp), BF16_NP)
    xch[:, :, 1:H + 1, 1:W + 1] = x_core.transpose(0, 3, 1, 2)
    xch = np.concatenate(
        [xch.reshape(NI * C * Hp, Wp), np.zeros((1, Wp), BF16_NP)], axis=0)

    x_px = np.ascontiguousarray(
        x_core.reshape(NI, H, NH, PW, C).transpose(3, 0, 1, 2, 4)
    ).reshape(PW, NRH, C).astype(BF16_NP)

    um = (ur_core[..., 0] <= FIRE_RATE).astype(np.float32)
    um_p = _to_rh(um, NI, H, NH).astype(BF16_NP)

    pre = (_maxpool3(x_core[..., 3]) > ALIVE_THR).astype(np.float32)
    plx_p = _to_rh(pre, NI, H, NH).astype(BF16_NP)

    return {
        "xch": xch,
        "xpx": x_px,
        "um": um_p,
        "plx": plx_p,
        "b01": b01,
        "b2": b2,
        "w1d": w1,
        "SEd": np.eye(PW, k=-1, dtype=np.float32).astype(BF16_NP),
        "SWd": np.eye(PW, k=1, dtype=np.float32).astype(BF16_NP),
    }


def unpack_xno(dev, NI, H, W):
    """[PW, NRH, C] device layout -> [NI, H, W, C] float32."""
    NH = W // PW
    return np.ascontiguousarray(
        dev.astype(np.float32).reshape(PW, NI, H, NH, C).transpose(1, 2, 3, 0, 4)
    ).reshape(NI, H, W, C)


def unpack_dxo(dev, NI, H, W, TR=16):
    """[NST, PW, 2048] channel-major packed dx -> [NI, H, W, C] float32."""
    NSI = H // TR // 2            # super-tiles per image
    d = dev.astype(np.float32).reshape(NI, NSI, 4, 32, 2, 2, 2, 2, PW)
    # [img, st, g, cc, ti, q, r2, h, w] -> [img, st, ti, q, g, r2, h, w, c]
    d = d[:, :, :, :C].transpose(0, 1, 4, 5, 2, 6, 7, 8, 3)
    return np.ascontiguousarray(d).reshape(NI, H, W, C)


@functools.lru_cache(maxsize=2)
def _cached_program(NI, H, W, TR):
    return build_program(NI, H, W, TR=TR)


def kernel(x, update_rand, w0, w1):
    x = np.asarray(x, np.float32)
    update_rand = np.asarray(update_rand, np.float32)
    B, H, W, _ = x.shape
    NI = B // N_CORES
    b01, b2, w1f = make_weights(w0, w1)

    nc = _cached_program(NI, H, W, 16)
    in_maps = [
        host_inputs(x[i * NI:(i + 1) * NI], update_rand[i * NI:(i + 1) * NI],
                    b01, b2, w1f, H, W)
        for i in range(N_CORES)
    ]
    res = run_bass_kernel_spmd(nc, in_maps, core_ids=list(range(N_CORES)))
    global LAST_RESULTS
    LAST_RESULTS = res
    x_new = np.concatenate(
        [unpack_xno(r["xno"], NI, H, W) for r in res.results], axis=0)
    dx = np.concatenate(
        [unpack_dxo(r["dxo"], NI, H, W) for r in res.results], axis=0)
    return x_new, dx


# revision 23
# speedup vs baseline: 2.2587x; 1.3644x over previous
"""Trainium2 Bass kernel for nn_CAModel (neural cellular automaton step).

Computation (per image, fp32):
  pre_life = maxpool3x3(x[...,3]) > 0.1        (HOST: exact fp32)
  gx, gy   = depthwise 3x3 sobel convs of x
  perc     = interleave([x, gx, gy])            # [H,W,48]
  h        = relu(perc @ w0)                    # [H,W,128]
  dx       = h @ w1                             # [H,W,16]
  x_mid    = x + dx * (update_rand <= 0.5)      (update mask from HOST)
  life     = pre_life & (maxpool3x3(x_mid[...,3]) > 0.1)
  x_new    = x_mid * life
  returns (x_new, dx)

Mapping: 8 NeuronCores, data-parallel over batch (2 images/core).

Device pipeline per 16-row tile (2 psum-halves q of 4 row-pair groups):
  fc0: sobel folded into weights; two K-stacked matmuls per row-pair
       (taps dx=0,1 stacked on 96 partitions, tap dx=2 separate) instead
       of three -> 2 streamed columns/pixel on the PE.
  relu: split across ACT and DVE engines (psum -> bf16 sbuf).
  fc1: w1-stationary (16-col weight loads), 4 row-pair groups packed
       into one PSUM tile at col-group offsets 0/32/64/96.
  dx transpose: channel-major [16,512] slabs -> pixel-major via HWDGE
       xbar DMA transpose (bf16).
  tail (gpsimd): x_mid = x + dx*um; alpha extract; per-image maxpool
       (vertical on DVE, horizontal via bf16 shift matmuls on PE),
       life mult, x_new store.  All I/O in bf16.
"""

import functools
import os
import sys

import numpy as np

_TRN_REPO = os.environ.get("TRN_RL_REPO", "/opt/trn_rl_repo")
if _TRN_REPO not in sys.path:
    sys.path.insert(0, _TRN_REPO)

import concourse.bass as bass
import concourse.bacc as bacc
import concourse.tile as tile
from concourse import mybir
from concourse.bass_utils import run_bass_kernel_spmd

F32 = mybir.dt.float32
BF16 = mybir.dt.bfloat16
BF16_NP = mybir.dt.np(mybir.dt.bfloat16)

C = 16          # channels
HID = 128       # hidden dim
PW = 128        # partitions used as w-position within a half
N_CORES = 8
FIRE_RATE = 0.5
ALIVE_THR = 0.1

LAST_RESULTS = None  # BassKernelResults of the most recent kernel() call


# ---------------------------------------------------------------------------
# device program
# ---------------------------------------------------------------------------

def build_program(NI, H, W, TR=16):
    """Build the Bass program for one core processing NI images of HxW."""
    NH = W // PW                  # halves per row (2)
    assert W % PW == 0 and H % TR == 0 and TR == 16 and NH == 2
    Hp, Wp = H + 2, W + 2
    NRH = NI * H * NH             # total (img,row,half) count
    NT = NI * H // TR             # tiles

    nc = bacc.Bacc(trn_type="TRN2")

    xch = nc.dram_tensor("xch", [NI * Hp, C, Wp], BF16, kind="ExternalInput")
    xpx_d = nc.dram_tensor("xpx", [PW, NRH, C], BF16, kind="ExternalInput")
    um_d = nc.dram_tensor("um", [PW, NRH], BF16, kind="ExternalInput")
    plx_d = nc.dram_tensor("plx", [PW, NRH], BF16, kind="ExternalInput")
    bw_d = nc.dram_tensor("bw", [3, 48, HID], BF16, kind="ExternalInput")
    w1_d = nc.dram_tensor("w1d", [HID, C], BF16, kind="ExternalInput")
    se_d = nc.dram_tensor("SEd", [PW, PW], BF16, kind="ExternalInput")
    sw_d = nc.dram_tensor("SWd", [PW, PW], BF16, kind="ExternalInput")
    dxo = nc.dram_tensor("dxo", [NT // 2, PW, 4 * 512], BF16,
                         kind="ExternalOutput")
    xno = nc.dram_tensor("xno", [PW, NRH, C], BF16, kind="ExternalOutput")

    with tile.TileContext(nc) as tc:
        _emit(tc, locals())
    nc.compile()
    return nc


def _emit(tc, t):
    nc = tc.nc
    NI, H, W, TR = t["NI"], t["H"], t["W"], t["TR"]
    NH, Hp, Wp, NRH, NT = t["NH"], t["Hp"], t["Wp"], t["NRH"], t["NT"]
    xch, xpx_d, um_d, plx_d = t["xch"], t["xpx_d"], t["um_d"], t["plx_d"]
    bw_d, w1_d, se_d, sw_d = t["bw_d"], t["w1_d"], t["se_d"], t["sw_d"]
    dxo, xno = t["dxo"], t["xno"]
    HNH = H * NH                  # per-image rh span (512)
    AL = mybir.AluOpType
    Relu = mybir.ActivationFunctionType.Relu

    from contextlib import ExitStack
    ctx = ExitStack()
    with ctx:
        singles = ctx.enter_context(tc.tile_pool(name="singles", bufs=1))
        xc_pool = ctx.enter_context(tc.tile_pool(name="xc", bufs=4))
        hs_pool = ctx.enter_context(tc.tile_pool(name="hs", bufs=8))
        xp_pool = ctx.enter_context(tc.tile_pool(name="xp", bufs=2))
        dxc_pool = ctx.enter_context(tc.tile_pool(name="dxc", bufs=2))
        dxt_pool = ctx.enter_context(tc.tile_pool(name="dxt", bufs=2))
        dxm_pool = ctx.enter_context(tc.tile_pool(name="dxm", bufs=2))
        xns_pool = ctx.enter_context(tc.tile_pool(name="xns", bufs=2))
        ps_h = ctx.enter_context(tc.tile_pool(name="ps_h", bufs=5, space="PSUM"))
        ps_dx = ctx.enter_context(tc.tile_pool(name="ps_dx", bufs=2, space="PSUM"))
        ps_scr = ctx.enter_context(tc.tile_pool(name="ps_scr", bufs=1, space="PSUM"))

        # ---- constants / weights / masks ----
        b_sb = singles.tile([48, 3, HID], BF16)
        nc.sync.dma_start(out=b_sb, in_=bw_d.ap().rearrange("t k m -> k t m"))
        w1_sb = singles.tile([HID, C], BF16)
        nc.sync.dma_start(out=w1_sb, in_=w1_d.ap())
        se_sb = singles.tile([PW, PW], BF16)
        nc.sync.dma_start(out=se_sb, in_=se_d.ap())
        sw_sb = singles.tile([PW, PW], BF16)
        nc.sync.dma_start(out=sw_sb, in_=sw_d.ap())
        um_sb = singles.tile([PW, NRH], BF16)
        nc.sync.dma_start(out=um_sb, in_=um_d.ap())
        plx_sb = singles.tile([PW, NRH], BF16)
        nc.sync.dma_start(out=plx_sb, in_=plx_d.ap())

        # ---- residents / mask scratch ----
        xmid = singles.tile([PW, NRH, C], BF16)
        vm = singles.tile([PW, HNH], BF16)
        vm2 = singles.tile([PW, HNH], BF16)
        m3 = singles.tile([PW, HNH], BF16)
        life = singles.tile([PW, HNH], BF16)
        seam = singles.tile([PW, 2 * H], BF16)
        nc.vector.memset(seam, 0.0)

        # ---- PE pre-sync dummies (touch each DMA'd matmul operand once) ----
        scr = ps_scr.tile([PW, 2], F32, tag="scr")
        nc.tensor.matmul(out=scr, lhsT=b_sb[:, 0, :], rhs=b_sb[:, 0, 0:2],
                         start=True, stop=True)
        scr2f = ps_scr.tile([PW, 2], F32, tag="scr")
        nc.tensor.matmul(out=scr2f[0:C, :], lhsT=w1_sb, rhs=w1_sb[:, 0:2],
                         start=True, stop=True)
        nc.tensor.matmul(out=scr, lhsT=se_sb, rhs=se_sb[:, 0:2],
                         start=True, stop=True)
        nc.tensor.matmul(out=scr, lhsT=sw_sb, rhs=sw_sb[:, 0:2],
                         start=True, stop=True)

        # ---- prime ps_dx garbage lanes so full-tile reads are defined ----
        for _ in range(2):
            pd0 = ps_dx.tile([PW, 512], F32, tag="dxc")
            nc.vector.memset(pd0, 0.0)

        def half_slice(tile_, p0, cnt, hf):
            return tile_[p0:p0 + cnt, :].rearrange(
                "p (r h) -> p r h", h=NH)[:, :, hf]

        def mask_phase(img):
            S = img * HNH

            def A(a, b):  # alpha channel of x_mid, strided view
                return xmid[:, S + a:S + b, 3]

            # vertical 3-max (rows clamped at image edge); row step in rh = NH
            nc.vector.tensor_tensor(
                out=vm[:, NH:], in0=A(NH, HNH), in1=A(0, HNH - NH), op=AL.max)
            nc.vector.tensor_copy(out=vm[:, 0:NH], in_=A(0, NH))
            nc.vector.tensor_tensor(
                out=vm2[:, 0:HNH - NH], in0=vm[:, 0:HNH - NH],
                in1=A(NH, HNH), op=AL.max)
            nc.vector.tensor_copy(out=vm2[:, HNH - NH:], in_=vm[:, HNH - NH:])

            # horizontal 3-max: PE shift-permutation matmuls (bf16) + seams
            pse = ps_h.tile([PW, HNH], F32, tag="psh")
            nc.tensor.matmul(out=pse, lhsT=se_sb, rhs=vm2,
                             start=True, stop=True)
            psw = ps_h.tile([PW, HNH], F32, tag="psh")
            nc.tensor.matmul(out=psw, lhsT=sw_sb, rhs=vm2,
                             start=True, stop=True)
            nc.vector.tensor_tensor(out=m3, in0=vm2, in1=pse, op=AL.max)
            nc.vector.tensor_tensor(out=m3, in0=m3, in1=psw, op=AL.max)

            # seam fixes between the two w-halves (zero elsewhere is
            # harmless for the >0.1 threshold)
            nc.sync.dma_start(out=seam[127:128, 0:H],
                              in_=half_slice(vm2, 0, 1, 1))
            nc.vector.tensor_tensor(
                out=half_slice(m3, 96, 32, 0), in0=half_slice(m3, 96, 32, 0),
                in1=seam[96:128, 0:H], op=AL.max)
            nc.sync.dma_start(out=seam[0:1, H:2 * H],
                              in_=half_slice(vm2, 127, 1, 0))
            nc.vector.tensor_tensor(
                out=half_slice(m3, 0, 32, 1), in0=half_slice(m3, 0, 32, 1),
                in1=seam[0:32, H:2 * H], op=AL.max)

            # life = (m3 > thr) * pre_life
            nc.vector.scalar_tensor_tensor(
                out=life, in0=m3, scalar=ALIVE_THR,
                in1=plx_sb[:, S:S + HNH], op0=AL.is_gt, op1=AL.mult)

            # x_new = x_mid * life, in 4 chunks alternating DVE/GPSIMD
            CH = HNH // 4
            for chk in range(4):
                rr = S + chk * CH
                xns = xns_pool.tile([PW, CH, C], BF16)
                eng = nc.vector if chk % 2 == 0 else nc.gpsimd
                eng.tensor_tensor(
                    out=xns, in0=xmid[:, rr:rr + CH, :],
                    in1=life[:, chk * CH:(chk + 1) * CH, None].to_broadcast(
                        [PW, CH, C]),
                    op=AL.mult)
                nc.scalar.dma_start(out=xno.ap()[:, rr:rr + CH, :], in_=xns)

        # ================= main loop (2-tile super-tiles) =================
        NST = NT // 2

        def emit_loads(st):
            rh_s = st * 2 * TR * NH
            xp = xp_pool.tile([PW, 2 * TR * NH, C], BF16)
            nc.sync.dma_start(
                out=xp, in_=xpx_d.ap()[:, rh_s:rh_s + 2 * TR * NH, :])
            xcs = []
            for ti in range(2):
                tt = 2 * st + ti
                img, t_in = divmod(tt, H // TR)
                a = t_in * TR
                # x in conv layout, partitions (dy, c)
                xc2 = xc_pool.tile([48, TR, Wp], BF16)
                src = bass.AP(
                    tensor=xch.ap().tensor,
                    offset=((img * Hp + a) * C) * Wp,
                    ap=[[C * Wp, 3], [Wp, C], [C * Wp, TR], [1, Wp]])
                nc.sync.dma_start(out=xc2, in_=src)
                xcs.append(xc2)
            return xp, xcs

        pending_mask = None
        ld = emit_loads(0)
        for st in range(NST):
            xp, xcs = ld
            rh_s = st * 2 * TR * NH
            dxcm = dxc_pool.tile([PW, 2, 2, 512], BF16)  # [p, ti, q, f]
            ncp = 0
            for ti in range(2):
                xc2 = xcs[ti]

                # ---- fc0: 3 streamed matmuls per row-pair group ----
                pshs = [None] * 8
                hsbs = [None] * 8
                for q in range(2):
                    for g in range(4):
                        pp = q * 4 + g
                        psh = ps_h.tile([HID, 512], F32, tag="psh")
                        pshs[pp] = psh
                        for dxi in range(3):
                            nc.tensor.matmul(
                                out=psh, lhsT=b_sb[:, dxi, :],
                                rhs=xc2[:, 2 * pp:2 * pp + 2, dxi:dxi + W],
                                start=(dxi == 0), stop=(dxi == 2))
                    for g in range(4):
                        pp = q * 4 + g
                        hsb = hs_pool.tile([HID, 512], BF16)
                        hsbs[pp] = hsb
                        if pp % 2 == 0:
                            nc.scalar.activation(
                                out=hsb, in_=pshs[pp], func=Relu)
                        else:
                            nc.vector.tensor_scalar(
                                out=hsb, in0=pshs[pp], scalar1=0.0,
                                scalar2=None, op0=AL.max)

                # ---- fc1: w1 stationary, 4 col-groups run concurrently ----
                for q in range(2):
                    pd = ps_dx.tile([PW, 512], F32, tag="dxc")
                    for g in range(4):
                        pp = q * 4 + g
                        nc.tensor.matmul(
                            out=pd[32 * g:32 * g + 16, :], lhsT=w1_sb,
                            rhs=hsbs[pp], start=True, stop=True,
                            skip_group_check=True, tile_position=(0, 32 * g))
                    if ncp % 2 == 0:
                        nc.vector.tensor_copy(out=dxcm[:, ti, q, :], in_=pd)
                    else:
                        nc.scalar.copy(out=dxcm[:, ti, q, :], in_=pd)
                    ncp += 1

            # prefetch next super-tile's inputs ahead of the transpose on
            # the sync queue so they are never blocked behind it
            if st + 1 < NST:
                ld = emit_loads(st + 1)

            # one xbar transpose for the whole super-tile:
            # [128(g,cc), 2048] -> [128w, 16(ti,q,r2,h), 128(g,cc)]
            dxt = dxt_pool.tile([PW, 16, PW], BF16)
            nc.sync.dma_start(out=dxt, in_=dxcm, transpose=True)
            nc.sync.dma_start(out=dxo.ap()[st], in_=dxcm)

            # ---- tail: x_mid = x + dx*um, alpha extract ----
            for ti in range(2):
                for q in range(2):
                    j0 = ti * 8 + q * 4
                    rh0 = rh_s + (ti * 2 + q) * TR
                    dxv = dxt[:, j0:j0 + 4, :].rearrange(
                        "p j (g cc) -> p g j cc", g=4)[:, :, :, 0:C]
                    umv = um_sb[:, rh0:rh0 + TR].rearrange(
                        "p (g j) -> p g j", g=4)[:, :, :, None].to_broadcast(
                        [PW, 4, 4, C])
                    dxm = dxm_pool.tile([PW, TR, C], F32)
                    nc.gpsimd.tensor_tensor(
                        out=dxm.rearrange("p (g j) c -> p g j c", g=4),
                        in0=dxv, in1=umv, op=AL.mult)
                    lq = (ti * 2 + q) * TR
                    nc.gpsimd.tensor_tensor(
                        out=xmid[:, rh0:rh0 + TR, :],
                        in0=xp[:, lq:lq + TR, :], in1=dxm, op=AL.add)

            # mask phase deferred one super-tile so its inputs are ready
            # the moment its instructions reach the engine queues
            if pending_mask is not None:
                mask_phase(pending_mask)
                pending_mask = None
            if (2 * st + 1) % (H // TR) == H // TR - 1:
                pending_mask = (2 * st + 1) // (H // TR)
        if pending_mask is not None:
            mask_phase(pending_mask)


# ---------------------------------------------------------------------------
# host side
# ---------------------------------------------------------------------------

def _sobel():
    kx = np.outer([1.0, 2.0, 1.0], [-1.0, 0.0, 1.0]) / 8.0
    ky = kx.T
    return kx, ky


def make_weights(w0, w1):
    """Fold sobel taps into fc0 -> Bw [3(dx), 48(dy*16+c), 128]."""
    kx, ky = _sobel()
    w0 = np.asarray(w0, np.float32)          # [48, 128]
    W0x = w0[0::3]                           # [16, 128]
    W0gx = w0[1::3]
    W0gy = w0[2::3]
    Bw = np.zeros((3, 48, HID), np.float32)
    for dy in range(3):
        for dxi in range(3):
            m = kx[dy, dxi] * W0gx + ky[dy, dxi] * W0gy
            if dy == 1 and dxi == 1:
                m = m + W0x
            Bw[dxi, dy * C:(dy + 1) * C, :] = m
    return Bw.astype(BF16_NP), np.asarray(w1, BF16_NP)


def _maxpool3(a):
    """3x3 max pool, stride 1, -inf padding. a: [NI, H, W]."""
    p = np.pad(a, ((0, 0), (1, 1), (1, 1)), constant_values=-np.inf)
    v = np.maximum(np.maximum(p[:, :-2, :], p[:, 1:-1, :]), p[:, 2:, :])
    return np.maximum(np.maximum(v[:, :, :-2], v[:, :, 1:-1]), v[:, :, 2:])


def _to_rh(a, NI, H, NH):
    """[NI, H, W] -> [PW, NI*H*NH] pixel-slab layout."""
    return np.ascontiguousarray(
        a.reshape(NI, H, NH, PW).transpose(3, 0, 1, 2)).reshape(PW, NI * H * NH)


def host_inputs(x_core, ur_core, bw, w1, H, W):
    """Build the per-core input map from [NI,H,W,C] x and [NI,H,W,1] rand."""
    NI = x_core.shape[0]
    NH = W // PW
    Hp, Wp = H + 2, W + 2
    NRH = NI * H * NH

    xch = np.zeros((NI, Hp, C, Wp), BF16_NP)
    xch[:, 1:H + 1, :, 1:W + 1] = x_core.transpose(0, 1, 3, 2)
    xch = xch.reshape(NI * Hp, C, Wp)

    x_px = np.ascontiguousarray(
        x_core.reshape(NI, H, NH, PW, C).transpose(3, 0, 1, 2, 4)
    ).reshape(PW, NRH, C).astype(BF16_NP)

    um = (ur_core[..., 0] <= FIRE_RATE).astype(np.float32)
    um_p = _to_rh(um, NI, H, NH).astype(BF16_NP)

    pre = (_maxpool3(x_core[..., 3]) > ALIVE_THR).astype(np.float32)
    plx_p = _to_rh(pre, NI, H, NH).astype(BF16_NP)

    return {
        "xch": xch,
        "xpx": x_px,
        "um": um_p,
        "plx": plx_p,
        "bw": bw,
        "w1d": w1,
        "SEd": np.eye(PW, k=-1, dtype=np.float32).astype(BF16_NP),
        "SWd": np.eye(PW, k=1, dtype=np.float32).astype(BF16_NP),
    }


def unpack_xno(dev, NI, H, W):
    """[PW, NRH, C] device layout -> [NI, H, W, C] float32."""
    NH = W // PW
    return np.ascontiguousarray(
        dev.astype(np.float32).reshape(PW, NI, H, NH, C).transpose(1, 2, 3, 0, 4)
    ).reshape(NI, H, W, C)


def unpack_dxo(dev, NI, H, W, TR=16):
    """[NST, PW, 2048] channel-major packed dx -> [NI, H, W, C] float32."""
    NSI = H // TR // 2            # super-tiles per image
    d = dev.astype(np.float32).reshape(NI, NSI, 4, 32, 2, 2, 2, 2, PW)
    # [img, st, g, cc, ti, q, r2, h, w] -> [img, st, ti, q, g, r2, h, w, c]
    d = d[:, :, :, :C].transpose(0, 1, 4, 5, 2, 6, 7, 8, 3)
    return np.ascontiguousarray(d).reshape(NI, H, W, C)


@functools.lru_cache(maxsize=2)
def _cached_program(NI, H, W, TR):
    return build_program(NI, H, W, TR=TR)


def kernel(x, update_rand, w0, w1):
    x = np.asarray(x, np.float32)
    update_rand = np.asarray(update_rand, np.float32)
    B, H, W, _ = x.shape
    NI = B // N_CORES
    bw, w1f = make_weights(w0, w1)

    nc = _cached_program(NI, H, W, 16)
    in_maps = [
        host_inputs(x[i * NI:(i + 1) * NI], update_rand[i * NI:(i + 1) * NI],
                    bw, w1f, H, W)
        for i in range(N_CORES)
    ]
    res = run_bass_kernel_spmd(nc, in_maps, core_ids=list(range(N_CORES)))
    global LAST_RESULTS
    LAST_RESULTS = res
    x_new = np.concatenate(
        [unpack_xno(r["xno"], NI, H, W) for r in res.results], axis=0)
    dx = np.concatenate(
        [unpack_dxo(r["dxo"], NI, H, W) for r in res.results], axis=0)
    return x_new, dx
